# revision 18
# baseline (speedup 1.0000x reference)
"""DenoiseGAT Trainium2 kernel v2: 8-core data-parallel, PE-centric GAT combine.

Per core: 256 polygons x 64 nodes = 16384 nodes = 128 node-tiles of 128
(2 polygons per tile). Activations h kept feature-major [128, 2t+k, 128]
(f = k*128+p, n = t*128+c). Per tile:
  - proj/skip/scores via matmuls with the h-slice as the stationary operand
    (node-major psum out), scores shifted via constant cyclic-shift matmuls;
  - softmax in node-partition layout (small free sizes);
  - attention applied as matmuls with per-(head,half) banded matrices built
    by a diagonal-scatter DMA into a DRAM band image (3-dim APs), loaded back
    contiguously;
  - skip/bias/temb folded into the same psum accumulation;
  - ELU on staged [128,2048] spans, split across ACT/DVE/Pool;
  - node-major output transposed back to feature-major with dma_start_transpose.
"""

import os
import numpy as np
import ml_dtypes
from contextlib import ExitStack

KDUMP = int(os.environ.get("KDUMP", "0"))

import concourse.bass as bass
import concourse.tile as tile
import concourse.tile_utils as tile_utils
from concourse import bacc, mybir
from concourse.ap import AP
from concourse.bass_utils import run_bass_kernel_spmd

tile_utils.max_sbuf_usage = 208 * 1024

F32 = mybir.dt.float32
BF16 = mybir.dt.bfloat16
ALU = mybir.AluOpType
ACTF = mybir.ActivationFunctionType

NCORES = 8
B, V = 2048, 64
HID, TDIM = 256, 128
BC = B // NCORES             # 256 polygons / core
N = BC * V                   # 16384 nodes / core
NT = N // 128                # 128 node-tiles
G2 = 16                      # softmax group (tiles)
CG = 8                       # Ac load group
EG = 8                       # ELU group (tiles) -> [128, 2048] spans
HG = 16                      # transpose group (tiles)
PIMG = NT * 4 * 66           # band image partition pitch (elems)
PSRC = NT * 4 * 3            # alpha buf partition pitch (elems)


def _ablk(asrc, atgt):
    NH, FO = asrc.shape
    out = np.zeros((NH * FO, 2 * NH), np.float32)
    for h in range(NH):
        out[h * FO:(h + 1) * FO, h] = asrc[h]
        out[h * FO:(h + 1) * FO, NH + h] = atgt[h]
    return out


def _bf(a):
    return np.ascontiguousarray(np.asarray(a, np.float32).astype(ml_dtypes.bfloat16))


def _f32(a):
    return np.ascontiguousarray(np.asarray(a, np.float32))


def half3(a):
    """(256, X) host -> (128, 2, X): tile[:, k, :] == rows 128k:128k+128."""
    a = np.asarray(a)
    return np.ascontiguousarray(a.reshape(2, 128, a.shape[1]).transpose(1, 0, 2))


def build(weights):
    nc = bacc.Bacc("TRN2", target_bir_lowering=False, debug=False,
                   enable_asserts=False, num_devices=NCORES)
    w = weights

    def inl(name, arr):
        return nc.inline_tensor(np.ascontiguousarray(arr), name=name).ap()

    half = TDIM // 2
    freqs = np.exp(-np.log(10000.0) * np.arange(half, dtype=np.float32) / (half - 1))
    fr2 = np.stack([np.concatenate([freqs, freqs]),
                    np.concatenate([np.zeros(half, np.float32),
                                    np.full(half, np.pi / 2, np.float32)])])

    W0 = _f32(w["W0"]); sk0 = _f32(w["skip0"])
    ab0 = _ablk(_f32(w["asrc0"]), _f32(w["atgt0"]))
    c_fr2 = inl("fr2", fr2.astype(np.float32))
    c_tW = inl("tW", _f32(w["tW"]))
    c_tb = inl("tb", _f32(w["tb"]).reshape(-1, 1))
    c_Wsum0t = inl("Wsum0t", W0[6:] + sk0[6:])           # (128, 256)
    c_W0ab = inl("W0ab", W0[6:] @ ab0)                   # (128, 8)
    c_W0f = inl("W0f", _bf(np.concatenate([W0[:6], sk0[:6]], 1)))   # (6, 512)
    c_Wa0 = inl("Wa0", _bf(W0[:6] @ ab0))                # (6, 8)
    c_eye8 = inl("eye8", np.eye(8, dtype=np.float32))
    c_if128 = inl("if128", np.eye(128, dtype=np.float32))
    c_i128 = inl("i128", _bf(np.eye(128, dtype=np.float32)))
    smm = np.zeros((128, 128), np.float32)
    smp = np.zeros((128, 128), np.float32)
    for hf in range(2):
        for v in range(64):
            smm[hf * 64 + (v - 1) % 64, hf * 64 + v] = 1.0
            smp[hf * 64 + (v + 1) % 64, hf * 64 + v] = 1.0
    c_smm = inl("smm", _bf(smm))
    c_smp = inl("smp", _bf(smp))
    ones2 = np.zeros((2, 128), np.float32)
    ones2[0, :64] = 1.0
    ones2[1, 64:] = 1.0
    c_ones2 = inl("ones2", _bf(ones2))
    c_onesr = inl("onesr", _bf(np.ones((1, 128), np.float32)))

    LW, LWA, LBR = {}, {}, {}
    for i in (1, 2):
        Wl = _f32(w[f"W{i}"]); skl = _f32(w[f"skip{i}"])
        abl = _ablk(_f32(w[f"asrc{i}"]), _f32(w[f"atgt{i}"]))
        LW[i] = inl(f"W{i}f", half3(_bf(np.concatenate([Wl, skl], 1))))   # [128,2,512]
        LWA[i] = inl(f"Wa{i}", half3(_bf(Wl @ abl)))                      # [128,2,8]
        LBR[i] = inl(f"b{i}r", _bf(np.tile(_f32(w[f"b{i}"]).reshape(1, -1), (1, 2))))  # [1,512]
    c_b0r = inl("b0r", _bf(np.tile(_f32(w["b0"]).reshape(1, -1), (1, 2))))
    W3 = _f32(w["W3"])
    ab3 = _ablk(_f32(w["asrc3"]), _f32(w["atgt3"]))
    c_W3 = inl("W3f", half3(_bf(W3)))                                     # [128,2,256]
    c_Wa3 = inl("Wa3", half3(_bf(W3 @ ab3)))                              # [128,2,2]
    c_b3r = inl("b3r", _bf(np.tile(_f32(w["b3"]).reshape(1, -1), (1, 2))))
    c_h1W = inl("h1Wf", half3(_bf(_f32(w["h1W"]))))
    c_h1b = inl("h1bc", half3(_f32(w["h1b"]).reshape(-1, 1)))
    c_h2W = inl("h2Wf", half3(_bf(_f32(w["h2W"]))))
    c_h2b = inl("h2bc", _f32(w["h2b"]).reshape(-1, 1))

    h0T = nc.dram_tensor("h0T", [6, N], BF16, kind="ExternalInput").ap()
    tp = nc.dram_tensor("tp", [2, BC], F32, kind="ExternalInput").ap()
    yT = nc.dram_tensor("yT", [2, N], F32, kind="ExternalOutput").ap()
    hdump = adump = None
    if KDUMP:
        hdump = nc.dram_tensor("hdump", [128, 2 * (N // 128) * 128], BF16,
                               kind="ExternalOutput").ap()
        adump = nc.dram_tensor("adump", [128, (N // 128) * 12], BF16,
                               kind="ExternalOutput").ap()

    def apx(t_ap, off, dims):
        return AP(t_ap.tensor, t_ap.offset + off, [list(d) for d in dims])

    with tile.TileContext(nc) as tc, ExitStack() as ctx:
        P = ctx.enter_context(tc.tile_pool(name="pers", bufs=1))
        WP = ctx.enter_context(tc.tile_pool(name="wts", bufs=1))
        DR = ctx.enter_context(tc.tile_pool(name="dram", bufs=1, space="DRAM"))
        PP = ctx.enter_context(tc.tile_pool(name="pp", bufs=3, space="PSUM"))
        PO = ctx.enter_context(tc.tile_pool(name="po", bufs=2, space="PSUM"))
        PG = ctx.enter_context(tc.tile_pool(name="pg", bufs=2, space="PSUM"))
        PSC = ctx.enter_context(tc.tile_pool(name="pssc", bufs=1, space="PSUM"))
        SG = ctx.enter_context(tc.tile_pool(name="sg", bufs=2))
        CB = ctx.enter_context(tc.tile_pool(name="cb", bufs=3))
        PJ = ctx.enter_context(tc.tile_pool(name="pj", bufs=3))

        TT = nc.vector.tensor_tensor
        STT = nc.vector.scalar_tensor_tensor
        TS = nc.vector.tensor_scalar

        h = P.tile([128, 2 * NT, 128], BF16, tag="h")
        halpha = [P.tile([128, NT, 4, 3], BF16, tag=f"alpha{i}", name=f"alpha{i}")
                  for i in range(2)]
        img2 = [DR.tile([128, PIMG], BF16, tag=f"img{i}", name=f"img{i}")
                for i in range(2)]

        def load(c_ap, tag):
            t = WP.tile(list(c_ap.shape), c_ap.dtype, tag=tag)
            nc.sync.dma_start(t[:], c_ap)
            return t

        # ---------- weights (startup-critical first) ----------
        t_h0 = P.tile([6, N], BF16, tag="h0")
        nc.sync.dma_start(t_h0[:], h0T)
        t_Wa0 = load(c_Wa0, "Wa0")
        t_smm = load(c_smm, "smm")
        t_smp = load(c_smp, "smp")
        t_i128 = load(c_i128, "i128")
        t_ones2 = load(c_ones2, "ones2")
        t_if128 = load(c_if128, "if128")
        t_onesr = load(c_onesr, "onesr")
        t_W0f = load(c_W0f, "W0f")
        t_b0r = load(c_b0r, "b0r")
        t_W = {1: load(LW[1], "W1"), 2: load(LW[2], "W2"), 3: load(c_W3, "W3")}
        t_Wa = {1: load(LWA[1], "Wa1"), 2: load(LWA[2], "Wa2"), 3: load(c_Wa3, "Wa3")}
        t_br = {0: t_b0r, 1: load(LBR[1], "b1r"), 2: load(LBR[2], "b2r"),
                3: load(c_b3r, "b3r")}

        # ---------- zero band image ----------
        zt = CB.tile([128, 8 * 264], BF16, tag="Ac", name="zt", bufs=3)
        nc.vector.memset(zt[:], 0.0)
        def zero_img(i):
            for q in range(4):
                dimg = apx(img2[i][:], q * 32 * 264,
                           [[PIMG, 128], [8 * 264, 4], [1, 8 * 264]])
                simg = apx(zt[:], 0, [[8 * 264, 128], [0, 4], [1, 8 * 264]])
                nc.sync.dma_start(dimg, simg)

        # ---------- temb ----------
        tembT = P.tile([TDIM, BC], F32, tag="tembT")
        t_fr2 = load(c_fr2, "fr2")
        t_tp = load(tp, "tp")
        ps_te = PSC.tile([TDIM, BC], F32, tag="psA")
        nc.tensor.matmul(ps_te[:], t_fr2[:], t_tp[:], start=True, stop=True)
        te_m = SG.tile([TDIM, BC], F32, tag="te_m", bufs=1)
        te_q = SG.tile([TDIM, BC], mybir.dt.int32, tag="te_q", bufs=1)
        nc.vector.tensor_scalar(te_q[:], ps_te[:], float(1.0 / (2 * np.pi)), None,
                                op0=ALU.mult)
        te_qf = SG.tile([TDIM, BC], F32, tag="te_qf", bufs=1)
        nc.vector.tensor_copy(te_qf[:], te_q[:])
        nc.vector.scalar_tensor_tensor(te_m[:], te_qf[:], float(-2 * np.pi), ps_te[:],
                                       op0=ALU.mult, op1=ALU.add)
        te_s = SG.tile([TDIM, BC], F32, tag="te_s", bufs=1)
        nc.scalar.activation(te_s[:], te_m[:], ACTF.Sin)
        t_tW = load(c_tW, "tW")
        t_tb = load(c_tb, "tb")
        ps_tm = PSC.tile([TDIM, BC], F32, tag="psA")
        nc.tensor.matmul(ps_tm[:], t_tW[:], te_s[:], start=True, stop=True)
        nc.scalar.activation(tembT[:], ps_tm[:], ACTF.Silu, bias=t_tb[:])

        # G0T [128, 2, 256] f32 ; s0g [8, 256] f32
        t_Ws0 = load(c_Wsum0t, "Ws0")
        G0T = P.tile([128, 2, BC], BF16, tag="G0T")
        for m in range(2):
            ps_g = PSC.tile([128, BC], F32, tag="psA")
            nc.tensor.matmul(ps_g[:], t_Ws0[:, m * 128:(m + 1) * 128], tembT[:],
                             start=True, stop=True)
            nc.vector.tensor_copy(G0T[:, m, :], ps_g[:])
        t_W0ab = load(c_W0ab, "W0ab")
        ps_sg = PSC.tile([8, BC], F32, tag="psA")
        nc.tensor.matmul(ps_sg[:], t_W0ab[:], tembT[:], start=True, stop=True)
        s0g = SG.tile([8, BC], F32, tag="s0g", bufs=1)
        nc.vector.tensor_copy(s0g[:], ps_sg[:])
        # transposes: G0n [128, 2, 256] bf16 ; s0gn [128, 2, 8] bf16
        t_eye8 = load(c_eye8, "eye8")
        G0n = P.tile([128, 2, 2, 128], BF16, tag="G0n")   # [g%128, ghalf, m, 128]
        for gh in range(2):
            for m in range(2):
                ps_t = PSC.tile([128, 128], BF16, tag="psA")
                nc.tensor.transpose(ps_t[:], G0T[:, m, gh * 128:(gh + 1) * 128],
                                    t_i128[:])
                nc.vector.tensor_copy(G0n[:, gh, m, :], ps_t[:])
        s0gn = P.tile([128, 2, 8], BF16, tag="s0gn")
        for gh in range(2):
            ps_t = PSC.tile([128, 8], F32, tag="psA")
            nc.tensor.matmul(ps_t[:], s0g[:, gh * 128:(gh + 1) * 128], t_eye8[:],
                             is_transpose=True, start=True, stop=True)
            nc.vector.tensor_copy(s0gn[:, gh, :], ps_t[:])
        # stage s0g/G0 through DRAM into tile-indexed layouts
        ds = DR.tile([256, 8], BF16, tag="ds")
        nc.sync.dma_start(
            apx(ds[:], 0, [[8, 128], [1024, 2], [1, 8]]),
            apx(s0gn[:], 0, [[16, 128], [8, 2], [1, 8]]))
        dg = DR.tile([256, 256], BF16, tag="dg")
        nc.sync.dma_start(
            apx(dg[:], 0, [[256, 128], [256 * 128, 2], [1, 256]]),
            apx(G0n[:], 0, [[512, 128], [256, 2], [1, 256]]))

        # ================= layers =================
        def _lparams(li):
            NH = 1 if li == 3 else 4
            return NH, (li == 0), (li == 3), halpha[li % 2], 2 * NH

        def hslice(t, k):
            return h[:, 2 * t + k, :]

        def phase_a(li, hl):
            NH, first, last, ab, SW = _lparams(li)
            PGW = G2 * SW          # scores region width
            shoff = PGW            # shifts region offset
            aoff = 2 * PGW         # alphaS region offset
            if True:
                for gg in range(hl * 4, hl * 4 + 4):
                    g0 = gg * G2
                    pg = PG.tile([128, 2 * PGW + G2 * 3 * NH], F32, tag="pg")
                    pga = pg[:]
                    for lg in range(G2):
                        t = g0 + lg
                        so = lg * SW
                        if first:
                            nc.tensor.matmul(
                                apx(pga, so, [[pga.ap[0][0], 128], [1, SW]]),
                                t_h0[:, t * 128:(t + 1) * 128],
                                t_Wa0[:], start=True, stop=True,
                                skip_group_check=True)
                        else:
                            for k in range(2):
                                nc.tensor.matmul(
                                    apx(pga, so, [[pga.ap[0][0], 128], [1, SW]]),
                                    hslice(t, k), t_Wa[li][:, k, :],
                                    start=(k == 0), stop=(k == 1),
                                    skip_group_check=True)
                    scG = SG.tile([128, G2, SW], BF16, tag="scG")
                    nc.vector.tensor_copy(scG[:], pga[:, 0:PGW].rearrange(
                        "p (g s) -> p g s", s=SW))
                    if first:
                        # add per-graph temb score offsets (broadcast from ds)
                        s0gb = SG.tile([128, G2, 8], BF16, tag="s0gb")
                        for hf2 in range(2):
                            nc.sync.dma_start(
                                apx(s0gb[:], 64 * hf2 * (G2 * 8),
                                    [[G2 * 8, 64], [8, G2], [1, 8]]),
                                apx(ds[:], (2 * g0 + hf2) * 8,
                                    [[0, 64], [16, G2], [1, 8]]))
                        TT(scG[:], scG[:], s0gb[:], op=ALU.add)
                    # shift matmuls (batched over group): rhs = src cols
                    rhs_src = apx(scG[:], 0, [[G2 * SW, 128], [SW, G2], [1, NH]])
                    for dlt, mat in ((0, t_smm), (1, t_smp)):
                        nc.tensor.matmul(
                            apx(pga, shoff + dlt * NH,
                                [[pga.ap[0][0], 128], [SW, G2], [1, NH]]),
                            mat[:], rhs_src, start=True, stop=True,
                            skip_group_check=True)
                    shG = SG.tile([128, G2, 2, NH], BF16, tag="shG")
                    nc.vector.tensor_copy(shG[:], pga[:, shoff:shoff + PGW].rearrange(
                        "p (g d j) -> p g d j", d=2, j=NH))
                    # E assembly: E[p, g, h, j]
                    E = SG.tile([128, G2, NH, 3], F32, tag="E")
                    tgt = apx(scG[:], NH, [[G2 * SW, 128], [SW, G2], [1, NH]])
                    TT(apx(E[:], 0, [[G2 * NH * 3, 128], [3 * NH, G2], [3, NH]]),
                       apx(shG[:], 0, [[G2 * 2 * NH, 128], [2 * NH, G2], [1, NH]]),
                       tgt, op=ALU.add)
                    TT(apx(E[:], 1, [[G2 * NH * 3, 128], [3 * NH, G2], [3, NH]]),
                       apx(scG[:], 0, [[G2 * SW, 128], [SW, G2], [1, NH]]),
                       tgt, op=ALU.add)
                    TT(apx(E[:], 2, [[G2 * NH * 3, 128], [3 * NH, G2], [3, NH]]),
                       apx(shG[:], NH, [[G2 * 2 * NH, 128], [2 * NH, G2], [1, NH]]),
                       tgt, op=ALU.add)
                    STT(E[:], E[:], 0.2, E[:], op0=ALU.mult, op1=ALU.max)
                    EX = SG.tile([128, G2, NH, 3], BF16, tag="EX")
                    nc.scalar.activation(EX[:], E[:], ACTF.Exp)
                    den = SG.tile([128, G2, NH], F32, tag="den")
                    TT(den[:], apx(EX[:], 0, [[G2 * NH * 3, 128], [3 * NH, G2], [3, NH]]),
                       apx(EX[:], 1, [[G2 * NH * 3, 128], [3 * NH, G2], [3, NH]]),
                       op=ALU.add)
                    TT(den[:], den[:],
                       apx(EX[:], 2, [[G2 * NH * 3, 128], [3 * NH, G2], [3, NH]]),
                       op=ALU.add)
                    rd = SG.tile([128, G2, NH], F32, tag="rd")
                    nc.vector.reciprocal(rd[:], den[:])
                    alph = SG.tile([128, G2, NH, 3], BF16, tag="alph")
                    TT(alph[:], EX[:], rd[:].unsqueeze(3).to_broadcast((128, G2, NH, 3)),
                       op=ALU.mult)
                    # alphaS via shift matmuls into psum region aoff
                    # j'=0: SMm @ alpha[..., 2] ; j'=1: I @ alpha[..., 1] ; j'=2: SMp @ alpha[..., 0]
                    for jp, (mat, jsrc) in enumerate(
                            ((t_smm, 2), (t_i128, 1), (t_smp, 0))):
                        nc.tensor.matmul(
                            apx(pga, aoff + jp,
                                [[pga.ap[0][0], 128], [3 * NH, G2], [3, NH]]),
                            mat[:],
                            apx(alph[:], jsrc,
                                [[G2 * NH * 3, 128], [3 * NH, G2], [3, NH]]),
                            start=True, stop=True, skip_group_check=True)
                    # copy alphaS -> layer alpha buf [128, g, 4, 3] (NH rows used)
                    nc.vector.tensor_copy(
                        apx(ab[:], g0 * 12, [[PSRC, 128], [12, G2], [1, 3 * NH]]),
                        pga[:, aoff:aoff + G2 * 3 * NH])
                # scatter for this half-layer
                img = img2[li % 2]
                g0 = hl * 64
                for hh in range(NH):
                    for hf in range(2):
                        d = apx(img[:], 64 * hf * PIMG + g0 * 264 + hh * 66,
                                [[PIMG + 1, 64], [264, 64], [1, 3]])
                        s = apx(ab[:], 64 * hf * PSRC + g0 * 12 + hh * 3,
                                [[PSRC, 64], [12, 64], [1, 3]])
                        nc.sync.dma_start(d, s)
                    dwa = apx(img[:], g0 * 264 + hh * 66 + 64,
                              [[64 * PIMG, 2], [264, 64], [1, 1]])
                    swa = apx(ab[:], g0 * 12 + hh * 3,
                              [[64 * PSRC, 2], [12, 64], [1, 1]])
                    nc.sync.dma_start(dwa, swa)
                    dwb = apx(img[:], 63 * PIMG + g0 * 264 + hh * 66 + 1,
                              [[64 * PIMG, 2], [264, 64], [1, 1]])
                    swb = apx(ab[:], 63 * PSRC + g0 * 12 + hh * 3 + 2,
                              [[64 * PSRC, 2], [12, 64], [1, 1]])
                    nc.sync.dma_start(dwb, swb)

        def phase_b(li, hl, part=None):
            NH, first, last, ab, SW = _lparams(li)
            img = img2[li % 2]
            lo = hl * (NT // CG // 2)
            hi = (hl + 1) * (NT // CG // 2)
            mid = (lo + hi) // 2
            if part == 0:
                hi = mid
            elif part == 1:
                lo = mid
            for cg in range(lo, hi):
                c0 = cg * CG
                Ac = CB.tile([128, CG * 264], BF16, tag="Ac", name="Ac", bufs=3)
                nc.sync.dma_start(
                    Ac[:], apx(img[:], c0 * 264, [[PIMG, 128], [1, CG * 264]]))
                for pr in range(CG // 2):
                    pp = PP.tile([128, 512], F32, tag="pp")
                    po = PO.tile([128, 512], F32, tag="po")
                    for tt_ in range(2):
                        t = c0 + pr * 2 + tt_
                        r = tt_ * 256
                        # proj psum
                        if first:
                            nc.tensor.matmul(pp[:, r:r + 256],
                                             t_h0[:, t * 128:(t + 1) * 128],
                                             t_W0f[:, 0:256], start=True, stop=True)
                        else:
                            for k in range(2):
                                nc.tensor.matmul(pp[:, r:r + 256], hslice(t, k),
                                                 t_W[li][:, k, 0:256],
                                                 start=(k == 0), stop=(k == 1))
                    pjT = PJ.tile([128, 512], BF16, tag="pjT")
                    if pr % 2 == 0:
                        nc.vector.tensor_copy(pjT[:], pp[:])
                    else:
                        nc.scalar.activation(pjT[:], pp[:], ACTF.Copy)
                    nc.tensor.matmul(po[:], t_onesr[:], t_br[li][:],
                                     start=True, stop=False, skip_group_check=True)
                    for tt_ in range(2):
                        t = c0 + pr * 2 + tt_
                        r = tt_ * 256
                        lg = t - c0
                        # skip accumulates
                        if first:
                            nc.tensor.matmul(po[:, r:r + 256],
                                             t_h0[:, t * 128:(t + 1) * 128],
                                             t_W0f[:, 256:512], start=False, stop=False,
                                             skip_group_check=True)
                        elif last:
                            for k in range(2):
                                nc.tensor.matmul(po[:, r + k * 128:r + k * 128 + 128],
                                                 hslice(t, k), t_i128[:],
                                                 start=False, stop=False,
                                                 skip_group_check=True)
                        else:
                            for k in range(2):
                                nc.tensor.matmul(po[:, r:r + 256], hslice(t, k),
                                                 t_W[li][:, k, 256:512],
                                                 start=False, stop=False,
                                                 skip_group_check=True)
                        # combine matmuls
                        for hf in range(2):
                            for hh in range(NH):
                                lhs = apx(Ac[:],
                                          64 * hf * (CG * 264) + lg * 264 + hh * 66 + 1,
                                          [[CG * 264, 64], [1, 64]])
                                fo = hh * 64 if not last else 0
                                fw = 64 if not last else 256
                                nc.tensor.matmul(
                                    apx(po[:], 64 * hf * 512 + r + fo,
                                        [[512, 64], [1, fw]]),
                                    lhs,
                                    apx(pjT[:], 64 * hf * 512 + r + fo,
                                        [[512, 64], [1, fw]]),
                                    start=False, stop=(hf == 1 and hh == NH - 1),
                                    skip_group_check=True)
                    # stage po -> pre
                    pe_i = ((c0 + pr * 2) % EG) // 2
                    if pe_i == 0:
                        phase_b.pre = CB.tile([128, EG * 256], BF16, tag="pre",
                                              name="pre", bufs=2)
                    pre = phase_b.pre
                    dst = pre[:, pe_i * 512:(pe_i + 1) * 512]
                    if last:
                        hb = (c0 + pr * 2) // HG % 2
                        hn = phase_b.hnm[hb]
                        lt = (c0 + pr * 2) % HG
                        if pr % 2 == 0:
                            nc.scalar.activation(hn[:, lt * 256:(lt + 2) * 256],
                                                 po[:], ACTF.Copy)
                        else:
                            nc.vector.tensor_copy(hn[:, lt * 256:(lt + 2) * 256], po[:])
                    else:
                        if pr % 2 == 0:
                            nc.scalar.activation(dst, po[:], ACTF.Copy)
                        else:
                            nc.vector.tensor_copy(dst, po[:])
                    if not last and pe_i == EG // 2 - 1:
                        # ELU on [128, EG*256] -> hnm slice
                        hb = (c0 + pr * 2) // HG % 2
                        hn = phase_b.hnm[hb]
                        l0 = ((c0 + pr * 2 + 2 - EG) % HG) * 256
                        e0 = c0 + pr * 2 + 2 - EG
                        if first:
                            g0g = CB.tile([128, EG, 256], BF16, tag="g0g",
                                          name="g0g", bufs=2)
                            for hf in range(2):
                                nc.sync.dma_start(
                                    apx(g0g[:], 64 * hf * (EG * 256),
                                        [[EG * 256, 64], [256, EG], [1, 256]]),
                                    apx(dg[:], (2 * e0 + hf) * 256,
                                        [[0, 64], [512, EG], [1, 256]]))
                            TT(pre[:], pre[:], g0g[:].rearrange("p a b -> p (a b)"),
                               op=ALU.add)
                        exv = CB.tile([128, EG * 256], BF16, tag="exv", name="exv", bufs=2)
                        nc.scalar.activation(exv[:], pre[:], ACTF.Exp)
                        rl = CB.tile([128, EG * 256], BF16, tag="rl", name="rl", bufs=2)
                        nc.gpsimd.tensor_scalar(rl[:], pre[:], 0.0, -1.0,
                                                op0=ALU.max, op1=ALU.add)
                        STT(hn[:, l0:l0 + EG * 256], exv[:], 1.0, rl[:],
                            op0=ALU.min, op1=ALU.add)
                # transpose back per HG tiles
                if (c0 + CG) % HG == 0:
                    tb = (c0 + CG) // HG - 1
                    hb = tb % 2
                    hn = phase_b.hnm[hb]
                    nc.sync.dma_start_transpose(
                        h[:, 32 * tb:32 * tb + 32, :], hn[:])

        phase_b.hnm = [P.tile([128, HG * 256], BF16, tag=f"hnm{i}", name=f"hnm{i}")
                     for i in range(2)]

        def dump(_li):
            if KDUMP == _li + 1:
                nc.sync.dma_start(hdump, h[:].rearrange("p m c -> p (m c)"))
                nc.sync.dma_start(
                    adump, halpha[_li % 2][:].rearrange("p g h j -> p (g h j)"))

        zero_img(0)
        phase_a(0, 0)
        phase_a(0, 1)
        zero_img(1)
        phase_b(0, 0)
        for _li in range(4):
            nxt = _li + 1
            phase_b(_li, 1, 0)
            if nxt < 4:
                phase_a(nxt, 0)
            phase_b(_li, 1, 1)
            dump(_li)
            if nxt < 4:
                phase_b(nxt, 0, 0)
                phase_a(nxt, 1)
                phase_b(nxt, 0, 1)

        # ---------- final MLP ----------
        t_h1W = load(c_h1W, "h1W")
        t_h1b = load(c_h1b, "h1b")
        t_h2W = load(c_h2W, "h2W")
        t_h2b = load(c_h2b, "h2b")
        for it in range(N // 512):
            pst = [PP.tile([128, 512], F32, tag="pp", name=f"mmh{m}") for m in range(1)]
            pst.append(PO.tile([128, 512], F32, tag="po", name="mmh1"))
            for m in range(2):
                for k in range(2):
                    rhs = apx(h[:], (8 * it + k) * 128,
                              [[2 * NT * 128, 128], [256, 4], [1, 128]])
                    nc.tensor.matmul(pst[m][:], t_h1W[:, k, m * 128:(m + 1) * 128],
                                     rhs, start=(k == 0), stop=(k == 1))
            h5 = PJ.tile([128, 2, 512], BF16, tag="h5", bufs=2)
            for m in range(2):
                nc.scalar.activation(h5[:, m, :], pst[m][:], ACTF.Silu,
                                     bias=t_h1b[:, m, :])
            ps2 = PSC.tile([2, 512], F32, tag="psA")
            for k in range(2):
                nc.tensor.matmul(ps2[:], t_h2W[:, k, :], h5[:, k, :],
                                 start=(k == 0), stop=(k == 1))
            yst = SG.tile([2, 512], F32, tag="yst", bufs=1)
            nc.vector.tensor_scalar(yst[:], ps2[:], t_h2b[:], None, op0=ALU.add)
            nc.sync.dma_start(yT[:, it * 512:(it + 1) * 512], yst[:])

    nc.compile()
    return nc


def kernel(**inputs):
    x = np.asarray(inputs["x"], np.float32)
    t = np.asarray(inputs["t"])
    nc = build(inputs)
    ph = np.arange(V, dtype=np.float32) * (2 * np.pi / V)
    posT = np.tile(np.stack([np.sin(ph), np.cos(ph), np.sin(2 * ph), np.cos(2 * ph)]),
                   (1, BC))
    in_maps = []
    for c in range(NCORES):
        xs = x[c * BC:(c + 1) * BC]
        xTs = np.ascontiguousarray(xs.reshape(N, 2).T)
        h0 = np.concatenate([xTs, posT], 0).astype(ml_dtypes.bfloat16)
        ts = t[c * BC:(c + 1) * BC].astype(np.float32)
        tps = np.ascontiguousarray(np.stack([ts, np.ones_like(ts)]))
        in_maps.append({"h0T": np.ascontiguousarray(h0), "tp": tps})
    res = run_bass_kernel_spmd(nc, in_maps, core_ids=list(range(NCORES)))
    outs = []
    for c in range(NCORES):
        yTs = res.results[c]["yT"]
        outs.append(yTs.T.reshape(BC, 2 * V).astype(np.float32))
    return np.concatenate(outs, 0)
